# revision 1
# baseline (speedup 1.0000x reference)
"""Channel-wise (XCA / XCiT-style) self-attention Trainium2 kernel.

Problem: x:(8,192,128,128) -> qkv proj -> per-head (d=24) channel attention
over N=16384 spatial positions with L2-normalized q,k -> out proj.

Sharding: data-parallel over batch B=8, one batch per NeuronCore (8 cores).
Each core runs an identical single-core program on its x[b] slice (C,N) and
produces out[b] (C,N); the host stacks the results.

Per-core dataflow (C=192 channels, N=16384, 8 heads x d=24, 2 head-groups
of 96 channels):
  pass 1 (streams x once):
    - v = Wv^T x      in (C,N) layout, kept resident in SBUF (fp16)
    - [q|k] = x^T Wqk in (N,2C) layout (fp16 tiles)
    - S^T[e,d] = k.q gram blocks + diag(q.q), diag(k.k) accumulated in PSUM
      (fp32) over all N (contraction on the partition axis)
  softmax phase (tiny): rq=1/max(sqrt(diag q.q),eps) (x temperature),
    rk likewise; S^T*rk -PE-transpose-> S*rq_t + block-diag mask -> softmax
    rows -> A; then the output projection is folded into the attention
    weights: Wt_g[e,c'] = sum_d A_g[d,e] Wp_g[d,c']  (one tiny matmul/group)
  pass 2: out[c',n] = sum_g sum_e Wt_g[e,c'] v_g[e,n]  (+ bias) -> DMA.

All heavy matmuls run in fp16 (1 cycle/row on the PE, 11-bit mantissa, fp32
PSUM accumulation). Host casts x/weights to fp16, which also halves the DMA
traffic; the output is written fp16 and upcast on the host. DMA is staged in
4096-wide tiles (8 KB descriptors), input on the SP HWDGE queue and output
on the ACT HWDGE queue.
"""

import numpy as np

B, C, HH, WW = 8, 192, 128, 128
N = HH * WW
NHEADS, DH, G, GC = 8, 24, 2, 96
NEG_BIG = -1.0e30

_BUILT = {}


def _patch_tile_drain():
    """The final TileContext drain carries one sem wait per live processor;
    this container's walrus codegen only accepts a single sync wait on the
    CTRL Drain. Split the waits across a chain of drains (1 wait each)."""
    import bass_rust
    import concourse.tile as tile
    from concourse.vector_clock import ScopedClock

    if getattr(tile.TileContext, "_drain_split_patch", False):
        return

    def _split_drain_and_barrier(self, tick_clock, wait_clock):
        nc = self.nc
        drain_bi = nc.sync.drain()
        wait_clock.add_sem_waits(
            drain_bi.ins, ScopedClock({None: tick_clock.global_clock})
        )
        inst = drain_bi.ins
        si = inst.sync_info
        if si is not None:
            waits = list(si.on_wait or [])
            ups = list(si.on_update or [])
            if len(waits) > 1:
                inst.sync_info = bass_rust.SyncInfo(on_wait=[waits[0]], on_update=[])
                for i, w in enumerate(waits[1:]):
                    extra = nc.sync.drain()
                    last = i == len(waits) - 2
                    extra.ins.sync_info = bass_rust.SyncInfo(
                        on_wait=[w], on_update=ups if last else []
                    )
        nc.all_engine_barrier()
        assert self.sems is not None
        popped = nc._tile_sem_poison_stack.pop()
        assert popped is self._sem_poison
        nc.clear_and_free_semaphores(list(self.sems.allocated().values()))
        nc.all_engine_barrier()

    tile.TileContext._drain_and_barrier = _split_drain_and_barrier
    tile.TileContext._drain_split_patch = True


def _split_excess_waits(nc, max_waits=1):
    """This container's walrus codegen accepts at most one sync wait per
    instruction. Move excess waits onto NoOp carriers inserted just before
    the instruction on the same engine (engine streams process waits in
    issue order, so this is semantics-preserving)."""
    import bass_rust
    from concourse import mybir

    for f in nc.m.functions:
        for bb in f.blocks:
            insts = bb.instructions
            if not any(
                getattr(i, "sync_info", None) is not None
                and i.sync_info.on_wait
                and len(list(i.sync_info.on_wait)) > max_waits
                for i in insts
            ):
                continue
            newlist = []
            for inst in insts:
                si = getattr(inst, "sync_info", None)
                if si is not None and si.on_wait:
                    waits = list(si.on_wait)
                    if len(waits) > max_waits:
                        keep = waits[-max_waits:]
                        for wi, w in enumerate(waits[: -max_waits]):
                            es = mybir.InstNoOp(
                                name=f"{inst.name}-xw{wi}", ins=[], outs=[]
                            )
                            es.engine = inst.engine
                            es.sync_info = bass_rust.SyncInfo(
                                on_wait=[w], on_update=[]
                            )
                            newlist.append(es)
                        inst.sync_info = bass_rust.SyncInfo(
                            on_wait=keep, on_update=list(si.on_update or [])
                        )
                newlist.append(inst)
            bb.instructions = newlist


def _build(n_total=N, split=True, repeat=1, stages=("v", "qk", "s", "sm", "p2")):
    """Build the single-core Bass program. Returns nc."""
    import contextlib as _ctxlib

    import concourse.bass as bass
    import concourse.tile as tile
    from concourse import mybir

    _patch_tile_drain()

    f32 = mybir.dt.float32
    f16 = mybir.dt.float16
    AFT = mybir.ActivationFunctionType
    ALU = mybir.AluOpType
    AX = mybir.AxisListType

    BW = 512  # compute block width
    CW = 4096 if n_total % 4096 == 0 else 512  # DMA staging chunk width
    NCH = n_total // CW  # staging chunks
    BPC = CW // BW  # compute blocks per chunk
    SUB = BW // 128  # 128-row n-subtiles per block

    nc = bass.Bass("TRN2", target_bir_lowering=False, debug=False)

    x = nc.dram_tensor("x", [C, n_total], f16, kind="ExternalInput").ap()
    wqk0 = nc.dram_tensor("wqk0", [128, 2 * C], f16, kind="ExternalInput").ap()
    wqk1 = nc.dram_tensor("wqk1", [64, 2 * C], f16, kind="ExternalInput").ap()
    wvt = nc.dram_tensor("wvt", [GC, G, C], f16, kind="ExternalInput").ap()
    bv16 = nc.dram_tensor("bv16", [GC, G], f16, kind="ExternalInput").ap()
    wp0 = nc.dram_tensor("wp0", [GC, C], f16, kind="ExternalInput").ap()
    wp1 = nc.dram_tensor("wp1", [GC, C], f16, kind="ExternalInput").ap()
    bqk = nc.dram_tensor("bqk", [1, 2 * C], f32, kind="ExternalInput").ap()
    bp = nc.dram_tensor("bp", [GC, G], f32, kind="ExternalInput").ap()
    tmp96 = nc.dram_tensor("tmp96", [GC, G], f32, kind="ExternalInput").ap()
    eye96 = nc.dram_tensor("eye96", [GC, GC], f32, kind="ExternalInput").ap()
    bmask = nc.dram_tensor("bmask", [GC, GC], f32, kind="ExternalInput").ap()
    out = nc.dram_tensor("out", [C, n_total], f16, kind="ExternalOutput").ap()

    with tile.TileContext(nc) as tc:
        with (
            tc.tile_pool(name="const", bufs=1) as const,
            tc.tile_pool(name="xp", bufs=2) as xp,
            tc.tile_pool(name="qkp", bufs=6) as qkp,
            tc.tile_pool(name="vres", bufs=1) as vres,
            tc.tile_pool(name="small", bufs=1) as small,
            tc.tile_pool(name="op", bufs=2) as op,
            tc.tile_pool(name="psA", bufs=3, space="PSUM") as psA,
            tc.tile_pool(name="psB", bufs=1, space="PSUM") as psB,
            tc.tile_pool(name="psS", bufs=1, space="PSUM") as psS,
        ):
            # --- constants into SBUF (first-use order) ---
            wqk0_sb = const.tile([128, 2 * C], f16)
            nc.sync.dma_start(wqk0_sb[:], wqk0)
            wqk1_sb = const.tile([64, 2 * C], f16)
            nc.sync.dma_start(wqk1_sb[:], wqk1)
            bqk_sb = const.tile([128, 2 * C], f32)
            nc.sync.dma_start(bqk_sb[:], bqk.to_broadcast([128, 2 * C]))

            # --- persistent tiles (x stays resident; pass 2 reads it) ---
            x0_sb = vres.tile([128, n_total], f16, tag="x0r", name="x0r")
            x1_sb = vres.tile([64, n_total], f16, tag="x1r", name="x1r")
            # S^T gram accumulators: [e(96), {S^T block | k-diag block}, 96]
            sg_ps = [
                psS.tile([GC, 2, GC], f32, tag=f"SG{g}", name=f"SG{g}")
                for g in range(G)
            ]
            qd_ps = [
                psS.tile([GC, GC], f32, tag=f"QD{g}", name=f"QD{g}")
                for g in range(G)
            ]

            _rep_cm = (
                tc.For_i(0, repeat, 1, hint_engines=tuple(nc.engines.keys()))
                if repeat > 1
                else _ctxlib.nullcontext()
            )
            with _rep_cm:
                # ---------------- pass 1 ----------------
                for ch in range(NCH):
                    cs = slice(ch * CW, (ch + 1) * CW)
                    if ch == 0:
                        # fine-grained first chunk so the PE starts ~8x sooner
                        for q in range(BPC):
                            qs = slice(q * BW, (q + 1) * BW)
                            nc.sync.dma_start(x0_sb[:, qs], x[0:128, qs])
                            nc.scalar.dma_start(x1_sb[:, qs], x[128:C, qs])
                    else:
                        nc.sync.dma_start(x0_sb[:, cs], x[0:128, cs])
                        nc.scalar.dma_start(x1_sb[:, cs], x[128:C, cs])
                    x0 = x0_sb[:, cs]
                    x1 = x1_sb[:, cs]

                    for bi in range(BPC):
                        blk = ch * BPC + bi
                        ns = slice(blk * BW, (blk + 1) * BW)
                        bs = slice(bi * BW, (bi + 1) * BW)

                        # q,k projection (N,2C layout) + gram accumulation
                        for j in range(SUB if "qk" in stages else 0):
                            first = blk == 0 and j == 0
                            last = blk == (n_total // BW) - 1 and j == SUB - 1
                            js = slice(bi * BW + j * 128, bi * BW + (j + 1) * 128)
                            pqk = psA.tile([128, 2 * C], f32, tag="A")
                            nc.tensor.matmul(
                                pqk[:], x0[:, js], wqk0_sb[:], start=True, stop=False
                            )
                            nc.tensor.matmul(
                                pqk[:], x1[:, js], wqk1_sb[:], start=False, stop=True
                            )
                            qk = qkp.tile([128, 2 * C], f16, tag="qk")
                            if "nocopy" not in stages:
                                nc.vector.tensor_tensor(
                                    out=qk[:], in0=pqk[:], in1=bqk_sb[:], op=ALU.add
                                )
                            else:
                                nc.vector.memset(qk[:, 0:1], 0.0)
                            # col layout: t*192 + g*96 + i  (t: 0=q 1=k)
                            qk4 = qk.rearrange("p (t g i) -> p t g i", t=2, g=G)
                            for g in range(G if "s" in stages else 0):
                                # S^T + k-diag: lhsT = k_g, rhs = [q_g | k_g]
                                nc.tensor.matmul(
                                    sg_ps[g][:],
                                    qk4[:, 1, g, :],
                                    qk4[:, :, g, :],
                                    start=first,
                                    stop=last,
                                    skip_group_check=True,
                                )
                                # q-diag: lhsT = q_g, rhs = q_g
                                nc.tensor.matmul(
                                    qd_ps[g][:],
                                    qk4[:, 0, g, :],
                                    qk4[:, 0, g, :],
                                    start=first,
                                    stop=last,
                                    skip_group_check=True,
                                )

                # --- softmax/pass-2 constants (not needed until pass 1 ends) ---
                wp0_sb = const.tile([GC, C], f16)
                nc.sync.dma_start(wp0_sb[:], wp0)
                wp1_sb = const.tile([GC, C], f16)
                nc.sync.dma_start(wp1_sb[:], wp1)
                bp_sb = const.tile([GC, G], f32)
                nc.sync.dma_start(bp_sb[:], bp)
                tmp96_sb = const.tile([GC, G], f32)
                nc.sync.dma_start(tmp96_sb[:], tmp96)
                eye_sb = const.tile([GC, GC], f32)
                nc.sync.dma_start(eye_sb[:], eye96)
                bmask_sb = const.tile([GC, GC], f32)
                nc.sync.dma_start(bmask_sb[:], bmask)

                # ---------------- softmax phase ----------------
                wt_sb = []
                if "sm" not in stages and "p2" in stages:
                    for g in range(G):
                        wtd = small.tile([GC, C], f16, tag=f"wt{g}", name=f"wtd{g}")
                        nc.vector.memset(wtd[:], 0.001)
                        wt_sb.append(wtd)
                for g in range(G if "sm" in stages else 0):
                    trash = small.tile([GC, GC], f32, tag="trash")
                    kss = small.tile([GC, 1], f32, tag=f"kss{g}")
                    nc.vector.tensor_mul(
                        out=trash[:], in0=sg_ps[g][:, 1, :], in1=eye_sb[:]
                    )
                    nc.vector.reduce_sum(out=kss[:], in_=trash[:], axis=AX.X)
                    trash2 = small.tile([GC, GC], f32, tag="trash")
                    qss = small.tile([GC, 1], f32, tag=f"qss{g}")
                    nc.vector.tensor_mul(out=trash2[:], in0=qd_ps[g][:], in1=eye_sb[:])
                    nc.vector.reduce_sum(out=qss[:], in_=trash2[:], axis=AX.X)
                    # r = 1 / max(sqrt(ss), eps)
                    for ss in (kss, qss):
                        nc.scalar.sqrt(ss[:], ss[:])
                        nc.vector.tensor_scalar_max(out=ss[:], in0=ss[:], scalar1=1e-12)
                        nc.vector.reciprocal(ss[:], ss[:])
                    # fold temperature into rq
                    nc.vector.tensor_tensor(
                        out=qss[:], in0=qss[:], in1=tmp96_sb[:, g, None], op=ALU.mult
                    )
                    # S^T scaled by rk (rows = e)
                    st_sb = small.tile([GC, GC], f32, tag="st")
                    nc.vector.tensor_scalar_mul(
                        out=st_sb[:], in0=sg_ps[g][:, 0, :], scalar1=kss[:]
                    )
                    # transpose -> S (rows = d)
                    ps_tr = psA.tile([GC, GC], f32, tag="A")
                    nc.tensor.transpose(ps_tr[:], st_sb[:], eye_sb[:])
                    s_sb = small.tile([GC, GC], f32, tag="s")
                    nc.vector.tensor_scalar_mul(
                        out=s_sb[:], in0=ps_tr[:], scalar1=qss[:]
                    )
                    nc.vector.tensor_tensor(
                        out=s_sb[:], in0=s_sb[:], in1=bmask_sb[:], op=ALU.add
                    )
                    # softmax rows
                    nmax = small.tile([GC, 1], f32, tag=f"nmax{g}")
                    nc.vector.reduce_max(
                        out=nmax[:], in_=s_sb[:], axis=AX.X, negate=True
                    )
                    e_sb = small.tile([GC, GC], f32, tag="e")
                    rsum = small.tile([GC, 1], f32, tag=f"rsum{g}")
                    nc.scalar.activation(
                        out=e_sb[:],
                        in_=s_sb[:],
                        func=AFT.Exp,
                        bias=nmax[:],
                        scale=1.0,
                        accum_out=rsum[:],
                    )
                    nc.vector.reciprocal(rsum[:], rsum[:])
                    a_sb = small.tile([GC, GC], f16, tag="a")
                    nc.vector.tensor_scalar_mul(
                        out=a_sb[:], in0=e_sb[:], scalar1=rsum[:]
                    )
                    # fold output projection: Wt_g[e,c'] = sum_d A_g[d,e] Wp_g[d,c']
                    ps_w = psA.tile([GC, C], f32, tag="A")
                    nc.tensor.matmul(
                        ps_w[:],
                        a_sb[:],
                        (wp0_sb if g == 0 else wp1_sb)[:],
                        start=True,
                        stop=True,
                    )
                    wt = small.tile([GC, C], f16, tag=f"wt{g}")
                    nc.scalar.activation(out=wt[:], in_=ps_w[:], func=AFT.Identity)
                    wt_sb.append(wt)

                # Wfused[c,c'] = sum_g sum_e Wv[c,96g+e] Wt_g[e,c']  and
                # bias_tot[c'] = sum_g Wt_g^T bv_g + bp  -> pass 2 is just
                # out = Wfused^T x + bias_tot.
                if "p2" in stages:
                    wvt_sb = const.tile([GC, G, C], f16, name="wvt_sb")
                    nc.sync.dma_start(wvt_sb[:], wvt)
                    bv16_sb = const.tile([GC, G], f16, name="bv16_sb")
                    nc.sync.dma_start(bv16_sb[:], bv16)
                    wf_sb = []
                    for kc, (p0, sz) in enumerate(((0, 128), (128, 64))):
                        ps_wf = psA.tile([128, C], f32, tag="A", name=f"pswf{kc}")
                        for g in range(G):
                            nc.tensor.matmul(
                                ps_wf[:sz, :],
                                wvt_sb[:, g, p0 : p0 + sz],
                                wt_sb[g][:],
                                start=(g == 0),
                                stop=(g == G - 1),
                            )
                        wf = small.tile([128, C], f16, tag=f"wf{kc}", name=f"wf{kc}")
                        nc.scalar.activation(
                            out=wf[:sz, :], in_=ps_wf[:sz, :], func=AFT.Identity
                        )
                        wf_sb.append(wf)
                    totb = small.tile([GC, G], f32, name="totb")
                    for mc in range(G):
                        msl = slice(mc * GC, (mc + 1) * GC)
                        pb = psB.tile([GC, 1], f32, tag="B", name=f"pb{mc}")
                        for g in range(G):
                            nc.tensor.matmul(
                                pb[:],
                                wt_sb[g][:, msl],
                                bv16_sb[:, g, None],
                                start=(g == 0),
                                stop=(g == G - 1),
                            )
                        nc.vector.tensor_tensor(
                            out=totb[:, mc, None],
                            in0=pb[:],
                            in1=bp_sb[:, mc, None],
                            op=ALU.add,
                        )

                # ---------------- pass 2 ----------------
                if "p2" in stages:
                    for ch in range(NCH):
                        cs = slice(ch * CW, (ch + 1) * CW)
                        ost = [
                            op.tile([GC, CW], f16, tag=f"ost{mc}", name=f"ost{mc}_{ch}")
                            for mc in range(G)
                        ]
                        for bi in range(BPC):
                            blk = ch * BPC + bi
                            ns = slice(blk * BW, (blk + 1) * BW)
                            bs = slice(bi * BW, (bi + 1) * BW)
                            for mc in range(G):
                                ms = slice(mc * GC, (mc + 1) * GC)
                                pout = psA.tile([GC, BW], f32, tag="A")
                                nc.tensor.matmul(
                                    pout[:],
                                    wf_sb[0][:, ms],
                                    x0_sb[:, ns],
                                    start=True,
                                    stop=False,
                                )
                                nc.tensor.matmul(
                                    pout[:],
                                    wf_sb[1][0:64, ms],
                                    x1_sb[:, ns],
                                    start=False,
                                    stop=True,
                                )
                                if mc == 0:
                                    nc.scalar.activation(
                                        out=ost[mc][:, bs],
                                        in_=pout[:],
                                        func=AFT.Identity,
                                        bias=totb[:, mc, None],
                                        scale=1.0,
                                    )
                                else:
                                    nc.vector.tensor_scalar_add(
                                        out=ost[mc][:, bs],
                                        in0=pout[:],
                                        scalar1=totb[:, mc, None],
                                    )
                        if ch == NCH - 1:
                            # stream the tail out per-block so the final drain
                            # overlaps compute instead of waiting on one big DMA
                            for q in range(BPC):
                                qs2 = slice(ch * CW + q * BW, ch * CW + (q + 1) * BW)
                                bs2 = slice(q * BW, (q + 1) * BW)
                                for mc in range(G):
                                    ms = slice(mc * GC, (mc + 1) * GC)
                                    eng = nc.scalar if mc == 0 else nc.sync
                                    eng.dma_start(out[ms, qs2], ost[mc][:, bs2])
                        else:
                            for mc in range(G):
                                ms = slice(mc * GC, (mc + 1) * GC)
                                eng = nc.scalar if mc == 0 else nc.sync
                                eng.dma_start(out[ms, cs], ost[mc][:])
                elif "od" in stages:
                    dummy_o = small.tile([GC, CW], f16, tag="dummy_o")
                    nc.vector.memset(dummy_o[:, 0:1], 0.0)
                    for ch in range(NCH):
                        cs = slice(ch * CW, (ch + 1) * CW)
                        for mc in range(G):
                            ms = slice(mc * GC, (mc + 1) * GC)
                            nc.scalar.dma_start(out[ms, cs], dummy_o[:])

    if split:
        _split_excess_waits(nc)
    return nc


def _host_aux(W_qkv, b_qkv, temperature, W_proj, b_proj):
    W_qkv = np.asarray(W_qkv, dtype=np.float32)
    b_qkv = np.asarray(b_qkv, dtype=np.float32)
    temperature = np.asarray(temperature, dtype=np.float32).reshape(NHEADS)
    W_proj = np.asarray(W_proj, dtype=np.float32)
    b_proj = np.asarray(b_proj, dtype=np.float32)

    f16 = np.float16
    aux = {
        "wqk0": np.ascontiguousarray(W_qkv[0:128, 0 : 2 * C]).astype(f16),
        "wqk1": np.ascontiguousarray(W_qkv[128:C, 0 : 2 * C]).astype(f16),
        "wvt": np.ascontiguousarray(
            W_qkv[:, 2 * C : 3 * C].T.reshape(G, GC, C).transpose(1, 0, 2)
        ).astype(f16),
        "wp0": np.ascontiguousarray(W_proj[0:GC, :]).astype(f16),
        "wp1": np.ascontiguousarray(W_proj[GC:C, :]).astype(f16),
        "bqk": np.ascontiguousarray(b_qkv[None, 0 : 2 * C]),
        "bv16": np.ascontiguousarray(
            np.stack(
                [b_qkv[2 * C + g * GC : 2 * C + (g + 1) * GC] for g in range(G)],
                axis=1,
            )
        ).astype(f16),
        "bp": np.ascontiguousarray(
            np.stack([b_proj[g * GC : (g + 1) * GC] for g in range(G)], axis=1)
        ),
        "tmp96": np.ascontiguousarray(
            np.stack(
                [np.repeat(temperature[4 * g : 4 * (g + 1)], DH) for g in range(G)],
                axis=1,
            )
        ),
        "eye96": np.eye(GC, dtype=np.float32),
        "bmask": np.where(
            np.kron(np.eye(4, dtype=bool), np.ones((DH, DH), dtype=bool)),
            np.float32(0.0),
            np.float32(NEG_BIG),
        ).astype(np.float32),
    }
    return aux


def kernel(x, W_qkv, b_qkv, temperature, W_proj, b_proj):
    from concourse.bass_utils import run_bass_kernel_spmd

    x = np.asarray(x, dtype=np.float32).reshape(B, C, N).astype(np.float16)
    aux = _host_aux(W_qkv, b_qkv, temperature, W_proj, b_proj)

    if "nc" not in _BUILT:
        _BUILT["nc"] = _build(N)
    nc = _BUILT["nc"]

    in_maps = [{"x": np.ascontiguousarray(x[b]), **aux} for b in range(B)]
    res = run_bass_kernel_spmd(nc, in_maps, core_ids=list(range(B)))
    out = np.stack([res.results[b]["out"] for b in range(B)], axis=0)
    return out.astype(np.float32).reshape(B, C, HH, WW)



# revision 4
# speedup vs baseline: 5.1378x; 5.1378x over previous
"""Channel-wise (XCA / XCiT-style) self-attention Trainium2 kernel.

Problem: x:(8,192,128,128) -> qkv proj -> per-head (d=24) channel attention
over N=16384 spatial positions with L2-normalized q,k -> out proj.

Sharding: data-parallel over batch B=8, one batch per NeuronCore (8 cores).
Each core runs an identical single-core program on its x[b] slice (C,N) and
produces out[b] (C,N); the host stacks the results.

Per-core dataflow (C=192 channels, N=16384, 8 heads x d=24, 2 head-groups
of 96 channels):
  pass 1 (streams x once):
    - [q|k] = x^T Wqk in (N,2C) layout (fp16 tiles). The qkv bias is folded
      into the projection by appending a constant-1 row to the lower x chunk
      (x1 is [65,N], row 64 = 1) and the bias row to wqk1, so the PSUM->SBUF
      evacuation is a pure dtype cast, alternated between the DVE and ACT
      engines.
    - S^T[e,d] = k.q gram blocks + diag(q.q), diag(k.k) accumulated in PSUM
      (fp32) over all N (contraction on the partition axis). The gram
      matmuls for subtile j are issued on the PE *after* the projection
      matmuls for subtile j+2 (software pipelining), so the PE never stalls
      waiting for the PSUM evacuation of its own just-produced tile; this
      keeps the tensor engine continuously busy (and therefore at its top
      p-state clock).
  softmax phase (tiny): rq=1/max(sqrt(diag q.q),eps) (x temperature),
    rk likewise; S^T*rk -PE-transpose-> S*rq_t + block-diag mask -> softmax
    rows -> A; then the output projection is folded into the attention
    weights: Wt_g[e,c'] = sum_d A_g[d,e] Wp_g[d,c']  (one tiny matmul/group)
  pass 2: out[c',n] = sum_g sum_e Wt_g[e,c'] v_g[e,n]  (+ bias) -> DMA.
    v is never materialized: Wfused[c,c'] = Wv Wt is built on-device and
    pass 2 is a single (C,C) matmul streaming the resident x.

All heavy matmuls run in fp16 (1 cycle/row on the PE, fp32 PSUM
accumulation). Host casts x/weights to fp16, which also halves the DMA
traffic; the output is written fp16 and upcast on the host. DMA is staged in
4096-wide chunks, input x0 on the SP HWDGE queue and x1 on the ACT HWDGE
queue (two rings in parallel); x is resident in SBUF as per-chunk tiles so
the repeat-loop iteration i+1 can prefetch chunk 0 while iteration i is
still in pass 2 on later chunks.
"""

import numpy as np

B, C, HH, WW = 8, 192, 128, 128
N = HH * WW
NHEADS, DH, G, GC = 8, 24, 2, 96
NEG_BIG = -1.0e30

_BUILT = {}


def _patch_tile_drain():
    """The final TileContext drain carries one sem wait per live processor;
    this container's walrus codegen only accepts a single sync wait on the
    CTRL Drain. Split the waits across a chain of drains (1 wait each)."""
    import bass_rust
    import concourse.tile as tile
    from concourse.vector_clock import ScopedClock

    if getattr(tile.TileContext, "_drain_split_patch", False):
        return

    def _split_drain_and_barrier(self, tick_clock, wait_clock):
        nc = self.nc
        drain_bi = nc.sync.drain()
        wait_clock.add_sem_waits(
            drain_bi.ins, ScopedClock({None: tick_clock.global_clock})
        )
        inst = drain_bi.ins
        si = inst.sync_info
        if si is not None:
            waits = list(si.on_wait or [])
            ups = list(si.on_update or [])
            if len(waits) > 1:
                inst.sync_info = bass_rust.SyncInfo(on_wait=[waits[0]], on_update=[])
                for i, w in enumerate(waits[1:]):
                    extra = nc.sync.drain()
                    last = i == len(waits) - 2
                    extra.ins.sync_info = bass_rust.SyncInfo(
                        on_wait=[w], on_update=ups if last else []
                    )
        nc.all_engine_barrier()
        assert self.sems is not None
        popped = nc._tile_sem_poison_stack.pop()
        assert popped is self._sem_poison
        nc.clear_and_free_semaphores(list(self.sems.allocated().values()))
        nc.all_engine_barrier()

    tile.TileContext._drain_and_barrier = _split_drain_and_barrier
    tile.TileContext._drain_split_patch = True


def _split_excess_waits(nc, max_waits=1):
    """This container's walrus codegen accepts at most one sync wait per
    instruction. Move excess waits onto NoOp carriers inserted just before
    the instruction on the same engine (engine streams process waits in
    issue order, so this is semantics-preserving)."""
    import bass_rust
    from concourse import mybir

    for f in nc.m.functions:
        for bb in f.blocks:
            insts = bb.instructions
            if not any(
                getattr(i, "sync_info", None) is not None
                and i.sync_info.on_wait
                and len(list(i.sync_info.on_wait)) > max_waits
                for i in insts
            ):
                continue
            newlist = []
            for inst in insts:
                si = getattr(inst, "sync_info", None)
                if si is not None and si.on_wait:
                    waits = list(si.on_wait)
                    if len(waits) > max_waits:
                        keep = waits[-max_waits:]
                        for wi, w in enumerate(waits[: -max_waits]):
                            es = mybir.InstNoOp(
                                name=f"{inst.name}-xw{wi}", ins=[], outs=[]
                            )
                            es.engine = inst.engine
                            es.sync_info = bass_rust.SyncInfo(
                                on_wait=[w], on_update=[]
                            )
                            newlist.append(es)
                        inst.sync_info = bass_rust.SyncInfo(
                            on_wait=keep, on_update=list(si.on_update or [])
                        )
                newlist.append(inst)
            bb.instructions = newlist


def _build(n_total=N, split=True, repeat=1, stages=("v", "qk", "s", "sm", "p2")):
    """Build the single-core Bass program. Returns nc."""
    import contextlib as _ctxlib

    import concourse.bass as bass
    import concourse.tile as tile
    from concourse import mybir

    _patch_tile_drain()

    f32 = mybir.dt.float32
    f16 = mybir.dt.float16
    AFT = mybir.ActivationFunctionType
    ALU = mybir.AluOpType
    AX = mybir.AxisListType

    BW = 512  # compute block width
    CW = 4096 if n_total % 4096 == 0 else 512  # DMA staging chunk width
    NCH = n_total // CW  # staging chunks
    BPC = CW // BW  # compute blocks per chunk
    SUB = BW // 128  # 128-row n-subtiles per block
    NSUB = n_total // 128  # total 128-row subtiles
    PIPE = 2  # software-pipeline lag (subtiles) between proj and gram

    nc = bass.Bass("TRN2", target_bir_lowering=False, debug=False)

    x0d = nc.dram_tensor("x0", [128, n_total], f16, kind="ExternalInput").ap()
    # x1 carries channels 128..191 plus a constant-1 row that folds the qkv
    # bias into the projection matmul.
    x1d = nc.dram_tensor("x1", [65, n_total], f16, kind="ExternalInput").ap()
    wqk0 = nc.dram_tensor("wqk0", [128, 2 * C], f16, kind="ExternalInput").ap()
    wqk1 = nc.dram_tensor("wqk1", [65, 2 * C], f16, kind="ExternalInput").ap()
    wvt = nc.dram_tensor("wvt", [GC, G, C], f16, kind="ExternalInput").ap()
    bv16 = nc.dram_tensor("bv16", [GC, G], f16, kind="ExternalInput").ap()
    wp0 = nc.dram_tensor("wp0", [GC, C], f16, kind="ExternalInput").ap()
    wp1 = nc.dram_tensor("wp1", [GC, C], f16, kind="ExternalInput").ap()
    bp = nc.dram_tensor("bp", [GC, G], f32, kind="ExternalInput").ap()
    tmp96 = nc.dram_tensor("tmp96", [GC, G], f32, kind="ExternalInput").ap()
    eye96 = nc.dram_tensor("eye96", [GC, GC], f32, kind="ExternalInput").ap()
    bmask = nc.dram_tensor("bmask", [GC, GC], f32, kind="ExternalInput").ap()
    out = nc.dram_tensor("out", [C, n_total], f16, kind="ExternalOutput").ap()

    with tile.TileContext(nc) as tc:
        with (
            tc.tile_pool(name="const", bufs=1) as const,
            tc.tile_pool(name="qkp", bufs=8) as qkp,
            tc.tile_pool(name="vres", bufs=1) as vres,
            tc.tile_pool(name="small", bufs=1) as small,
            tc.tile_pool(name="op", bufs=2) as op,
            tc.tile_pool(name="psA", bufs=5, space="PSUM") as psA,
            tc.tile_pool(name="psB", bufs=1, space="PSUM") as psB,
            tc.tile_pool(name="psS", bufs=1, space="PSUM") as psS,
        ):
            # --- constants into SBUF (first-use order) ---
            wqk0_sb = const.tile([128, 2 * C], f16)
            nc.sync.dma_start(wqk0_sb[:], wqk0)
            wqk1_sb = const.tile([65, 2 * C], f16)
            nc.sync.dma_start(wqk1_sb[:], wqk1)

            # --- persistent x tiles, one pair per chunk (pass 2 re-reads) ---
            x0_sb = [
                vres.tile([128, CW], f16, tag=f"x0r{ch}", name=f"x0r{ch}")
                for ch in range(NCH)
            ]
            x1_sb = [
                vres.tile([65, CW], f16, tag=f"x1r{ch}", name=f"x1r{ch}")
                for ch in range(NCH)
            ]
            # gram accumulators: [e(96), {S^T | k-diag | q-diag}, 96]
            # (one PSUM bank per group)
            sg_ps = [
                psS.tile([GC, 3, GC], f32, tag=f"SG{g}", name=f"SG{g}")
                for g in range(G)
            ]

            _rep_cm = (
                tc.For_i(0, repeat, 1, hint_engines=tuple(nc.engines.keys()))
                if repeat > 1
                else _ctxlib.nullcontext()
            )
            with _rep_cm:
                # ---------------- pass 1 ----------------
                # Software-pipelined: proj matmuls for subtile j are issued
                # PIPE subtiles ahead of the gram matmuls for subtile j, so
                # the PE never waits on the DVE/ACT PSUM evacuation.
                pend = []  # (qk_tile, j) awaiting gram emission

                def emit_gram(qk, j):
                    first = j == 0
                    last = j == NSUB - 1
                    # col layout: t*192 + g*96 + i  (t: 0=q 1=k)
                    qk4 = qk.rearrange("p (t g i) -> p t g i", t=2, g=G)
                    for g in range(G):
                        # S^T + k-diag: lhsT = k_g, rhs = [q_g | k_g]
                        nc.tensor.matmul(
                            sg_ps[g][:, 0:2, :],
                            qk4[:, 1, g, :],
                            qk4[:, :, g, :],
                            start=first,
                            stop=last,
                            skip_group_check=True,
                        )
                        # q-diag: lhsT = q_g, rhs = q_g
                        nc.tensor.matmul(
                            sg_ps[g][:, 2, :],
                            qk4[:, 0, g, :],
                            qk4[:, 0, g, :],
                            start=first,
                            stop=last,
                            skip_group_check=True,
                        )

                for ch in range(NCH):
                    if ch == 0:
                        # fine-grained first chunk so the PE starts ~8x sooner
                        for q in range(BPC):
                            qs = slice(q * BW, (q + 1) * BW)
                            nc.sync.dma_start(x0_sb[ch][:, qs], x0d[:, qs])
                            nc.scalar.dma_start(x1_sb[ch][:, qs], x1d[:, qs])
                    else:
                        cs = slice(ch * CW, (ch + 1) * CW)
                        nc.sync.dma_start(x0_sb[ch][:], x0d[:, cs])
                        nc.scalar.dma_start(x1_sb[ch][:], x1d[:, cs])

                    for bi in range(BPC):
                        for j in range(SUB if "qk" in stages else 0):
                            blk = ch * BPC + bi
                            jg = blk * SUB + j  # global subtile index
                            js = slice(bi * BW + j * 128, bi * BW + (j + 1) * 128)
                            pqk = psA.tile([128, 2 * C], f32, tag="A")
                            nc.tensor.matmul(
                                pqk[:],
                                x0_sb[ch][:, js],
                                wqk0_sb[:],
                                start=True,
                                stop=False,
                            )
                            nc.tensor.matmul(
                                pqk[:],
                                x1_sb[ch][:, js],
                                wqk1_sb[:],
                                start=False,
                                stop=True,
                            )
                            qk = qkp.tile([128, 2 * C], f16, tag="qk")
                            # pure cast (bias folded into the ones-row),
                            # alternated between DVE and ACT
                            if "nocopy" not in stages:
                                if jg % 2 == 0:
                                    nc.vector.tensor_copy(out=qk[:], in_=pqk[:])
                                else:
                                    nc.scalar.activation(
                                        out=qk[:], in_=pqk[:], func=AFT.Identity
                                    )
                            else:
                                nc.vector.memset(qk[:, 0:1], 0.0)
                            if "s" in stages:
                                pend.append((qk, jg))
                                if len(pend) > PIPE:
                                    emit_gram(*pend.pop(0))
                for qk, jg in pend:
                    emit_gram(qk, jg)
                pend = []

                # --- softmax/pass-2 constants (not needed until pass 1 ends) ---
                wp0_sb = const.tile([GC, C], f16)
                nc.sync.dma_start(wp0_sb[:], wp0)
                wp1_sb = const.tile([GC, C], f16)
                nc.sync.dma_start(wp1_sb[:], wp1)
                bp_sb = const.tile([GC, G], f32)
                nc.sync.dma_start(bp_sb[:], bp)
                tmp96_sb = const.tile([GC, G], f32)
                nc.sync.dma_start(tmp96_sb[:], tmp96)
                eye_sb = const.tile([GC, GC], f32)
                nc.sync.dma_start(eye_sb[:], eye96)
                bmask_sb = const.tile([GC, GC], f32)
                nc.sync.dma_start(bmask_sb[:], bmask)

                # ---------------- softmax phase ----------------
                wt_sb = []
                if "sm" not in stages and "p2" in stages:
                    for g in range(G):
                        wtd = small.tile([GC, C], f16, tag=f"wt{g}", name=f"wtd{g}")
                        nc.vector.memset(wtd[:], 0.001)
                        wt_sb.append(wtd)
                for g in range(G if "sm" in stages else 0):
                    trash = small.tile([GC, GC], f32, tag="trash")
                    kss = small.tile([GC, 1], f32, tag=f"kss{g}")
                    nc.vector.tensor_mul(
                        out=trash[:], in0=sg_ps[g][:, 1, :], in1=eye_sb[:]
                    )
                    nc.vector.reduce_sum(out=kss[:], in_=trash[:], axis=AX.X)
                    trash2 = small.tile([GC, GC], f32, tag="trash")
                    qss = small.tile([GC, 1], f32, tag=f"qss{g}")
                    nc.vector.tensor_mul(out=trash2[:], in0=sg_ps[g][:, 2, :], in1=eye_sb[:])
                    nc.vector.reduce_sum(out=qss[:], in_=trash2[:], axis=AX.X)
                    # r = 1 / max(sqrt(ss), eps)
                    for ss in (kss, qss):
                        nc.scalar.sqrt(ss[:], ss[:])
                        nc.vector.tensor_scalar_max(out=ss[:], in0=ss[:], scalar1=1e-12)
                        nc.vector.reciprocal(ss[:], ss[:])
                    # fold temperature into rq
                    nc.vector.tensor_tensor(
                        out=qss[:], in0=qss[:], in1=tmp96_sb[:, g, None], op=ALU.mult
                    )
                    # S^T scaled by rk (rows = e)
                    st_sb = small.tile([GC, GC], f32, tag="st")
                    nc.vector.tensor_scalar_mul(
                        out=st_sb[:], in0=sg_ps[g][:, 0, :], scalar1=kss[:]
                    )
                    # transpose -> S (rows = d)
                    ps_tr = psA.tile([GC, GC], f32, tag="A")
                    nc.tensor.transpose(ps_tr[:], st_sb[:], eye_sb[:])
                    s_sb = small.tile([GC, GC], f32, tag="s")
                    nc.vector.tensor_scalar_mul(
                        out=s_sb[:], in0=ps_tr[:], scalar1=qss[:]
                    )
                    nc.vector.tensor_tensor(
                        out=s_sb[:], in0=s_sb[:], in1=bmask_sb[:], op=ALU.add
                    )
                    # softmax rows
                    nmax = small.tile([GC, 1], f32, tag=f"nmax{g}")
                    nc.vector.reduce_max(
                        out=nmax[:], in_=s_sb[:], axis=AX.X, negate=True
                    )
                    e_sb = small.tile([GC, GC], f32, tag="e")
                    rsum = small.tile([GC, 1], f32, tag=f"rsum{g}")
                    nc.scalar.activation(
                        out=e_sb[:],
                        in_=s_sb[:],
                        func=AFT.Exp,
                        bias=nmax[:],
                        scale=1.0,
                        accum_out=rsum[:],
                    )
                    nc.vector.reciprocal(rsum[:], rsum[:])
                    a_sb = small.tile([GC, GC], f16, tag="a")
                    nc.vector.tensor_scalar_mul(
                        out=a_sb[:], in0=e_sb[:], scalar1=rsum[:]
                    )
                    # fold output projection: Wt_g[e,c'] = sum_d A_g[d,e] Wp_g[d,c']
                    ps_w = psA.tile([GC, C], f32, tag="A")
                    nc.tensor.matmul(
                        ps_w[:],
                        a_sb[:],
                        (wp0_sb if g == 0 else wp1_sb)[:],
                        start=True,
                        stop=True,
                    )
                    wt = small.tile([GC, C], f16, tag=f"wt{g}")
                    nc.scalar.activation(out=wt[:], in_=ps_w[:], func=AFT.Identity)
                    wt_sb.append(wt)

                # Wfused[c,c'] = sum_g sum_e Wv[c,96g+e] Wt_g[e,c']  and
                # bias_tot[c'] = sum_g Wt_g^T bv_g + bp  -> pass 2 is just
                # out = Wfused^T x + bias_tot.
                if "p2" in stages:
                    wvt_sb = const.tile([GC, G, C], f16, name="wvt_sb")
                    nc.sync.dma_start(wvt_sb[:], wvt)
                    bv16_sb = const.tile([GC, G], f16, name="bv16_sb")
                    nc.sync.dma_start(bv16_sb[:], bv16)
                    wf_sb = []
                    for kc, (p0, sz) in enumerate(((0, 128), (128, 64))):
                        ps_wf = psA.tile([128, C], f32, tag="A", name=f"pswf{kc}")
                        for g in range(G):
                            nc.tensor.matmul(
                                ps_wf[:sz, :],
                                wvt_sb[:, g, p0 : p0 + sz],
                                wt_sb[g][:],
                                start=(g == 0),
                                stop=(g == G - 1),
                            )
                        wf = small.tile([128, C], f16, tag=f"wf{kc}", name=f"wf{kc}")
                        nc.scalar.activation(
                            out=wf[:sz, :], in_=ps_wf[:sz, :], func=AFT.Identity
                        )
                        wf_sb.append(wf)
                    totb = small.tile([GC, G], f32, name="totb")
                    for mc in range(G):
                        msl = slice(mc * GC, (mc + 1) * GC)
                        pb = psB.tile([GC, 1], f32, tag="B", name=f"pb{mc}")
                        for g in range(G):
                            nc.tensor.matmul(
                                pb[:],
                                wt_sb[g][:, msl],
                                bv16_sb[:, g, None],
                                start=(g == 0),
                                stop=(g == G - 1),
                            )
                        nc.vector.tensor_tensor(
                            out=totb[:, mc, None],
                            in0=pb[:],
                            in1=bp_sb[:, mc, None],
                            op=ALU.add,
                        )

                # ---------------- pass 2 ----------------
                if "p2" in stages:
                    for ch in range(NCH):
                        cs = slice(ch * CW, (ch + 1) * CW)
                        ost = [
                            op.tile([GC, CW], f16, tag=f"ost{mc}", name=f"ost{mc}_{ch}")
                            for mc in range(G)
                        ]
                        for bi in range(BPC):
                            bs = slice(bi * BW, (bi + 1) * BW)
                            for mc in range(G):
                                ms = slice(mc * GC, (mc + 1) * GC)
                                pout = psA.tile([GC, BW], f32, tag="A")
                                nc.tensor.matmul(
                                    pout[:],
                                    wf_sb[0][:, ms],
                                    x0_sb[ch][:, bs],
                                    start=True,
                                    stop=False,
                                )
                                nc.tensor.matmul(
                                    pout[:],
                                    wf_sb[1][0:64, ms],
                                    x1_sb[ch][0:64, bs],
                                    start=False,
                                    stop=True,
                                )
                                if mc == 0:
                                    nc.scalar.activation(
                                        out=ost[mc][:, bs],
                                        in_=pout[:],
                                        func=AFT.Identity,
                                        bias=totb[:, mc, None],
                                        scale=1.0,
                                    )
                                else:
                                    nc.vector.tensor_scalar_add(
                                        out=ost[mc][:, bs],
                                        in0=pout[:],
                                        scalar1=totb[:, mc, None],
                                    )
                        if ch == NCH - 1:
                            # stream the tail out per-block so the final drain
                            # overlaps compute instead of waiting on one big DMA
                            for q in range(BPC):
                                qs2 = slice(ch * CW + q * BW, ch * CW + (q + 1) * BW)
                                bs2 = slice(q * BW, (q + 1) * BW)
                                for mc in range(G):
                                    ms = slice(mc * GC, (mc + 1) * GC)
                                    eng = nc.scalar if mc == 0 else nc.sync
                                    eng.dma_start(out[ms, qs2], ost[mc][:, bs2])
                        else:
                            for mc in range(G):
                                ms = slice(mc * GC, (mc + 1) * GC)
                                eng = nc.scalar if mc == 0 else nc.sync
                                eng.dma_start(out[ms, cs], ost[mc][:])
                elif "od" in stages:
                    dummy_o = small.tile([GC, CW], f16, tag="dummy_o")
                    nc.vector.memset(dummy_o[:, 0:1], 0.0)
                    for ch in range(NCH):
                        cs = slice(ch * CW, (ch + 1) * CW)
                        for mc in range(G):
                            ms = slice(mc * GC, (mc + 1) * GC)
                            nc.scalar.dma_start(out[ms, cs], dummy_o[:])

    if split:
        _split_excess_waits(nc)
    return nc


def _host_aux(W_qkv, b_qkv, temperature, W_proj, b_proj):
    W_qkv = np.asarray(W_qkv, dtype=np.float32)
    b_qkv = np.asarray(b_qkv, dtype=np.float32)
    temperature = np.asarray(temperature, dtype=np.float32).reshape(NHEADS)
    W_proj = np.asarray(W_proj, dtype=np.float32)
    b_proj = np.asarray(b_proj, dtype=np.float32)

    f16 = np.float16
    wqk1 = np.concatenate(
        [W_qkv[128:C, 0 : 2 * C], b_qkv[None, 0 : 2 * C]], axis=0
    )
    aux = {
        "wqk0": np.ascontiguousarray(W_qkv[0:128, 0 : 2 * C]).astype(f16),
        "wqk1": np.ascontiguousarray(wqk1).astype(f16),
        "wvt": np.ascontiguousarray(
            W_qkv[:, 2 * C : 3 * C].T.reshape(G, GC, C).transpose(1, 0, 2)
        ).astype(f16),
        "wp0": np.ascontiguousarray(W_proj[0:GC, :]).astype(f16),
        "wp1": np.ascontiguousarray(W_proj[GC:C, :]).astype(f16),
        "bv16": np.ascontiguousarray(
            np.stack(
                [b_qkv[2 * C + g * GC : 2 * C + (g + 1) * GC] for g in range(G)],
                axis=1,
            )
        ).astype(f16),
        "bp": np.ascontiguousarray(
            np.stack([b_proj[g * GC : (g + 1) * GC] for g in range(G)], axis=1)
        ),
        "tmp96": np.ascontiguousarray(
            np.stack(
                [np.repeat(temperature[4 * g : 4 * (g + 1)], DH) for g in range(G)],
                axis=1,
            )
        ),
        "eye96": np.eye(GC, dtype=np.float32),
        "bmask": np.where(
            np.kron(np.eye(4, dtype=bool), np.ones((DH, DH), dtype=bool)),
            np.float32(0.0),
            np.float32(NEG_BIG),
        ).astype(np.float32),
    }
    return aux


def make_in_maps(x, W_qkv, b_qkv, temperature, W_proj, b_proj):
    x = np.asarray(x, dtype=np.float32).reshape(B, C, N).astype(np.float16)
    aux = _host_aux(W_qkv, b_qkv, temperature, W_proj, b_proj)
    ones_row = np.ones((1, N), dtype=np.float16)
    return [
        {
            "x0": np.ascontiguousarray(x[b, 0:128]),
            "x1": np.ascontiguousarray(
                np.concatenate([x[b, 128:C], ones_row], axis=0)
            ),
            **aux,
        }
        for b in range(B)
    ]


def kernel(x, W_qkv, b_qkv, temperature, W_proj, b_proj):
    from concourse.bass_utils import run_bass_kernel_spmd

    if "nc" not in _BUILT:
        _BUILT["nc"] = _build(N)
    nc = _BUILT["nc"]

    in_maps = make_in_maps(x, W_qkv, b_qkv, temperature, W_proj, b_proj)
    res = run_bass_kernel_spmd(nc, in_maps, core_ids=list(range(B)))
    out = np.stack([res.results[b]["out"] for b in range(B)], axis=0)
    return out.astype(np.float32).reshape(B, C, HH, WW)
